# revision 1
# baseline (speedup 1.0000x reference)
"""Block-causal self-attention (SSMax) Trainium2 kernel.

Full inputs in, full output out. Sharding: 8 cores = 2 batches x 4 head
groups (3 heads each). Each core computes qkv for its head slice, the
block-causal attention for its 3 heads, and a partial c_proj product;
the host sums the 4 partials per batch.

Device-side layout notes (per core):
  - x is shipped pre-transposed: xt [768, 2048] so the tensor engine can
    contract over channels (K=partition) with natural DMA layouts.
  - c_attn slice shipped as wqkv [768, 576], column order
    [q_h0*, k_h0, q_h1*, k_h1, q_h2*, k_h2, v_h0, v_h1, v_h2] (64 cols
    each); q columns pre-scaled by s*log(T)/sqrt(hd) so softmax scaling
    is free.
  - Scores are computed transposed (ST[j, i] = k_j . q_i) so the exp'd
    tile is directly the stationary operand of the y matmul; an extra
    ones-column appended to v yields the softmax denominator in the same
    accumulation.
  - Softmax skips the max-subtraction pass: scores are ~N(0,1) for this
    problem so exp is fp32/bf16-safe.
"""

import numpy as np

T = 2048
C = 768
HEADS_PER_CORE = 3
HD = 64
NBLK = 64  # block-causal block size
KC = 6  # 768 / 128 contraction chunks
N_CORES = 8

_CACHE: dict = {}


def _build_bass():
    import concourse.bacc as bacc
    import concourse.mybir as mybir
    import concourse.tile as tile
    from concourse._compat import get_trn_type
    from concourse.masks import make_identity

    dt = mybir.dt
    f32 = dt.float32
    f32r = dt.float32r
    bf16 = dt.bfloat16
    EXP = mybir.ActivationFunctionType.Exp

    nc = bacc.Bacc(get_trn_type() or "TRN2", debug=False)
    xt_d = nc.dram_tensor("xt", [C, T], f32r, kind="ExternalInput")
    wqkv_d = nc.dram_tensor("wqkv", [C, 576], f32r, kind="ExternalInput")
    wproj_d = nc.dram_tensor("wproj", [256, C], f32r, kind="ExternalInput")
    out_d = nc.dram_tensor("out", [T, C], f32, kind="ExternalOutput")
    warm_d = nc.dram_tensor("warm", [128, 1], f32, kind="ExternalOutput")

    with tile.TileContext(nc) as tc:
        with (
            tc.tile_pool(name="persist", bufs=1) as persist,
            tc.tile_pool(name="ps_big", bufs=2, space="PSUM") as ps_big,
            tc.tile_pool(name="ps_st", bufs=2, space="PSUM") as ps_st,
            tc.tile_pool(name="ps_y", bufs=2, space="PSUM") as ps_y,
            tc.tile_pool(name="exp_pool", bufs=2) as exp_pool,
            tc.tile_pool(name="small", bufs=4) as small,
            tc.tile_pool(name="outst", bufs=3) as outst,
        ):
            xt_all = persist.tile([128, KC, T], f32r, tag="xt")
            w_all = persist.tile([128, KC, 576], f32r, tag="w")
            wp_all = persist.tile([128, 2, C], f32r, tag="wp")
            # wqkv column order (64 each): [q0,k0 | q1,k1 | q2,k2 | v0,v1 | v2].
            # The PE crashes if consecutive instructions use different base
            # partitions, so everything it touches is staged at base 0:
            # k_h and v1 are shifted down with SBUF->SBUF DMAs after the
            # qkv projection.
            qk0 = persist.tile([128, T], bf16, tag="qk0")  # [q0; k0]
            qk1 = persist.tile([128, T], bf16, tag="qk1")  # [q1; k1]
            qk2 = persist.tile([128, T], bf16, tag="qk2")  # [q2; k2]
            vst = persist.tile([128, T], bf16, tag="vst")  # [v0; v1]
            v2st = persist.tile([64, T], bf16, tag="v2")  # [v2]
            # k goes to rows 0:64 of its own tile; rows 64:128 of both the
            # k tiles and the q tiles are zeroed so score matmuls run with
            # K=128 (K=64 matmuls serialize LDWEIGHTS, costing 2x)
            kt0 = persist.tile([128, T], bf16, tag="kt0")
            kt1 = persist.tile([128, T], bf16, tag="kt1")
            kt2 = persist.tile([128, T], bf16, tag="kt2")
            v1t = persist.tile([64, T], bf16, tag="v1t")
            v_all = persist.tile([128, 16, 195], bf16, tag="v")
            # dims 192:256 stay zero so the second y-transpose window is a
            # full 128 columns (64-wide fp32 transposes crash the PE)
            y_all = persist.tile([128, 16, 256], f32, tag="y")
            yt_all = persist.tile([128, 2, T], f32r, tag="yt")
            id_bf = persist.tile([128, 128], bf16, tag="idb")
            id_f = persist.tile([128, 128], f32, tag="idf")

            make_identity(nc, id_bf)
            make_identity(nc, id_f)

            # ---- PE warm-up: dense dummy matmuls during the DMA prologue
            # keep the HAM clock-gate open so qkv starts at 2.4 GHz ----
            wsink = persist.tile([128, 1], f32, tag="wsink")
            for wi in range(100):
                pw = ps_y.tile([128, 128], f32, tag="py")
                nc.tensor.matmul(
                    pw[:, :], lhsT=id_bf[:, :], rhs=id_bf[:, :],
                    start=True, stop=True,
                )
                if wi == 99:
                    nc.vector.tensor_copy(out=wsink[:, :], in_=pw[:, 0:1])
            nc.sync.dma_start(out=warm_d[:, :], in_=wsink[:, :])

            # ---- loads ----
            for kc in range(KC):
                nc.sync.dma_start(
                    out=w_all[:, kc, :], in_=wqkv_d[128 * kc : 128 * kc + 128, :]
                )
            # wproj is host-padded to 256 rows (rows 192:256 zero) so both
            # slots DMA straight in; the zero rows pair with the zero-padded
            # yt slot 1 rows in the projection matmul
            nc.sync.dma_start(out=wp_all[:, 0, :], in_=wproj_d[0:128, :])
            nc.sync.dma_start(out=wp_all[:, 1, :], in_=wproj_d[128:256, :])
            nc.gpsimd.memset(y_all[:, :, 192:256], 0.0)
            for t4 in range(4):
                ts = slice(512 * t4, 512 * t4 + 512)
                for kc in range(KC):
                    nc.sync.dma_start(
                        out=xt_all[:, kc, ts],
                        in_=xt_d[128 * kc : 128 * kc + 128, ts],
                    )

            # ---- qkv projection: qkvT[m, t] chunks ----
            qkv_dst = [qk0, qk1, qk2, vst, v2st]
            for t4 in range(4):
                ts = slice(512 * t4, 512 * t4 + 512)
                for m in range(5):
                    rows = 128 if m < 4 else 64
                    ps = ps_big.tile([128, 512], f32, tag="ps")
                    for kc in range(KC):
                        nc.tensor.matmul(
                            ps[0:rows, :],
                            lhsT=w_all[:, kc, 128 * m : 128 * m + rows],
                            rhs=xt_all[:, kc, ts],
                            start=(kc == 0),
                            stop=(kc == KC - 1),
                        )
                    nc.vector.tensor_copy(
                        out=qkv_dst[m][0:rows, ts], in_=ps[0:rows, :]
                    )

            # ---- shift k_h / v1 to base partition 0 (SBUF->SBUF DMA),
            # then zero-pad the contraction dim of the score operands ----
            nc.sync.dma_start(out=kt0[0:64, :], in_=qk0[64:128, :])
            nc.sync.dma_start(out=kt1[0:64, :], in_=qk1[64:128, :])
            nc.sync.dma_start(out=kt2[0:64, :], in_=qk2[64:128, :])
            nc.sync.dma_start(out=v1t[:, :], in_=vst[64:128, :])
            for t_ in (kt0, kt1, kt2, qk0, qk1, qk2):
                nc.gpsimd.memset(t_[64:128, :], 0.0)

            # ---- v transpose into [token, head-dim] layout + ones column ----
            for tcn in range(16):
                tsl = slice(128 * tcn, 128 * tcn + 128)
                pv = ps_y.tile([128, 192], bf16, tag="py")
                nc.tensor.transpose(
                    pv[:, 0:64], vst[0:64, tsl], id_bf[0:64, 0:64]
                )
                nc.tensor.transpose(
                    pv[:, 64:128], v1t[0:64, tsl], id_bf[0:64, 0:64]
                )
                nc.tensor.transpose(
                    pv[:, 128:192], v2st[0:64, tsl], id_bf[0:64, 0:64]
                )
                vdst = v_all[:, tcn, :].rearrange("p (h e) -> p h e", e=65)
                nc.vector.tensor_copy(
                    out=vdst[:, :, 0:64],
                    in_=pv[:, 0:192].rearrange("p (h e) -> p h e", e=64),
                )
                nc.vector.memset(vdst[:, :, 64:65], 1.0)

            # ---- attention, group (ci) outer / head inner so the y rows of
            # each 512-token group complete together and projection can
            # overlap the next group's attention ----
            head_ops = [
                (kt0, qk0),
                (kt1, qk1),
                (kt2, qk2),
            ]
            for ci in range(4):
                i_base = 512 * ci
                for h in range(HEADS_PER_CORE):
                    k_sl, q_sl = head_ops[h]
                    # score tiles in pairs of j-chunks: two matmuls into one
                    # 2-bank psum, one wide exp (halves ACT instruction count)
                    ets = {}
                    npair = 2 * ci + 2
                    for p in range(npair):
                        ps = ps_st.tile([128, 1024], f32, tag="st")
                        et = exp_pool.tile([128, 1024], bf16, tag=f"p{p}")
                        exp_from = None  # start col of a pending fused exp
                        for half in range(2):
                            jc = 2 * p + half
                            m = jc - 4 * ci
                            i0 = 128 * m if m >= 0 else 0
                            lo = 512 * half
                            nc.tensor.matmul(
                                ps[:, lo + i0 : lo + 512],
                                lhsT=k_sl[:, 128 * jc : 128 * jc + 128],
                                rhs=q_sl[:, i_base + i0 : i_base + 512],
                                start=True,
                                stop=True,
                            )  # K=128 with zero-padded rows 64:128
                            if i0 == 0 and half == 0:
                                exp_from = 0  # may fuse with second half
                            elif i0 == 0 and exp_from == 0:
                                pass  # second half contiguous with first
                            else:
                                if exp_from is not None:
                                    nc.scalar.activation(
                                        et[:, exp_from:lo], ps[:, exp_from:lo], EXP
                                    )
                                exp_from = lo + i0
                            ets[jc] = et
                        nc.scalar.activation(
                            et[:, exp_from:1024], ps[:, exp_from:1024], EXP
                        )
                        for half in range(2):
                            jc = 2 * p + half
                            m = jc - 4 * ci
                            if m >= 0:
                                i0 = 512 * half + 128 * m
                                # block-causal: upper half-block keys masked
                                # for lower half-block queries
                                nc.vector.memset(et[64:128, i0 : i0 + 64], 0.0)
                    for r in range(4):
                        c = 4 * ci + r
                        py = ps_y.tile([128, 65], f32, tag="py")
                        for jc in range(c + 1):
                            lo = 512 * (jc & 1)
                            nc.tensor.matmul(
                                py[:, :],
                                lhsT=ets[jc][:, lo + 128 * r : lo + 128 * r + 128],
                                rhs=v_all[:, jc, 65 * h : 65 * h + 65],
                                start=(jc == 0),
                                stop=(jc == c),
                            )
                        rec = small.tile([128, 1], f32, tag="rec")
                        nc.vector.reciprocal(rec, py[:, 64:65])
                        nc.vector.tensor_scalar_mul(
                            y_all[:, c, 64 * h : 64 * h + 64], py[:, 0:64], rec
                        )

                # ---- y transpose + projection for this group's 4 t-chunks,
                # overlapping the next group's attention ----
                for r in range(4):
                    tcn = 4 * ci + r
                    tsl = slice(128 * tcn, 128 * tcn + 128)
                    pt = ps_big.tile([128, 512], f32, tag="ps")
                    nc.tensor.transpose(pt[:, 0:128], y_all[:, tcn, 0:128], id_f)
                    # window 128:256 is zero-padded beyond dim 192, keeping
                    # the transpose full-width and the result at base 0
                    nc.tensor.transpose(
                        pt[:, 128:256], y_all[:, tcn, 128:256], id_f
                    )
                    nc.vector.tensor_copy(out=yt_all[:, 0, tsl], in_=pt[:, 0:128])
                    nc.vector.tensor_copy(out=yt_all[:, 1, tsl], in_=pt[:, 128:256])
                    ot = outst.tile([128, C], f32, tag="ot")
                    for oc, ow in ((0, 512), (1, 256)):
                        pp = ps_big.tile([128, 512], f32, tag="ps")
                        osl = slice(512 * oc, 512 * oc + ow)
                        nc.tensor.matmul(
                            pp[:, 0:ow],
                            lhsT=yt_all[:, 0, tsl],
                            rhs=wp_all[:, 0, osl],
                            start=True,
                            stop=False,
                        )
                        nc.tensor.matmul(
                            pp[:, 0:ow],
                            lhsT=yt_all[:, 1, tsl],
                            rhs=wp_all[:, 1, osl],
                            start=False,
                            stop=True,
                        )
                        nc.vector.tensor_copy(out=ot[:, osl], in_=pp[:, 0:ow])
                    nc.sync.dma_start(out=out_d[tsl, :], in_=ot[:, :])

    nc.compile()
    return nc


def _get_nc():
    if "nc" not in _CACHE:
        _CACHE["nc"] = _build_bass()
    return _CACHE["nc"]


def make_in_maps(x, c_attn_w, c_proj_w, s):
    x = np.asarray(x, dtype=np.float32)
    c_attn_w = np.asarray(c_attn_w, dtype=np.float32)
    c_proj_w = np.asarray(c_proj_w, dtype=np.float32)
    s = np.asarray(s, dtype=np.float32)

    scale = np.float32(s[0] * np.log(T).astype(np.float32))
    f = np.float32(scale * np.float32(1.0 / np.sqrt(HD)))

    in_maps = []
    for b in range(2):
        xt = np.ascontiguousarray(x[b].T)  # [768, 2048]
        for g in range(4):
            h0, h1, h2 = 3 * g, 3 * g + 1, 3 * g + 2
            qrow = lambda h: c_attn_w[64 * h : 64 * h + 64] * f  # scaled q
            krow = lambda h: c_attn_w[C + 64 * h : C + 64 * h + 64]
            vrow = lambda h: c_attn_w[2 * C + 64 * h : 2 * C + 64 * h + 64]
            # column order [q0,k0 | q1,k1 | q2,k2 | v0,v1 | v2] (see device side)
            wsel = np.concatenate(
                [
                    qrow(h0), krow(h0),
                    qrow(h1), krow(h1),
                    qrow(h2), krow(h2),
                    vrow(h0), vrow(h1),
                    vrow(h2),
                ],
                axis=0,
            )  # [576, 768]
            wqkv = np.ascontiguousarray(wsel.T)  # [768, 576]
            wproj = np.zeros((256, C), np.float32)  # rows 192:256 stay zero
            wproj[0:192] = c_proj_w[:, 192 * g : 192 * g + 192].T
            in_maps.append({"xt": xt, "wqkv": wqkv, "wproj": wproj})
    return in_maps


def gather(results):
    out = np.empty((2, T, C), dtype=np.float32)
    for b in range(2):
        acc = results[4 * b]["out"].astype(np.float32)
        for g in range(1, 4):
            acc = acc + results[4 * b + g]["out"]
        out[b] = acc
    return out


def kernel(x, c_attn_w, c_proj_w, s):
    from concourse.bass_utils import run_bass_kernel_spmd

    nc = _get_nc()
    in_maps = make_in_maps(x, c_attn_w, c_proj_w, s)
    res = run_bass_kernel_spmd(nc, in_maps, list(range(N_CORES)))
    return gather(res.results)

